# revision 2
# baseline (speedup 1.0000x reference)
"""2-layer GCN (GCNConv x2 + log_softmax) on 8 trn2 NeuronCores via Bass/Tile.

Rewrite of the baseline with drastically fewer/bigger ops:
  - phase A (q = dis*(x@W1)) stores once via a single rearranged DMA
  - one slot DMA per pass; per-slot [P,1] indirect gathers (HW limit)
  - fused per-tile DVE math; per-group batched L2 matmuls
  - log_softmax tail with batched reductions

Table layout: T1 rows = node order with per-core TP padding
  (node n -> row (n//S)*TP + n%S); T2 rows = degree-perm order
  (node n -> row (n//S)*TP + invperm(n%S)).
"""

import numpy as np

import concourse.bass as bass
import concourse.mybir as mybir
import concourse.tile as tile
from concourse.masks import make_identity
from concourse.vector_clock import ScopedClock

P = 128
F1 = 16
F2 = 40
D = 512
GROUP = 2
N_NODES = 100000
N_CORES = 8

# ---------------------------------------------------------------------------
# workaround: this walrus build rejects >1 sync wait per instruction and the
# Drain opcode; spill extra waits onto single-wait nops.
_nop_counter = [0]


def _fresh_nop(engine, wait):
    _nop_counter[0] += 1
    nop = mybir.InstNoOp(name=f"WSPILL-{_nop_counter[0]}", ins=[], outs=[])
    nop.engine = engine
    nop.sync_info = mybir.SyncInfo(on_wait=[wait], on_update=[])
    return nop


def _split_multi_waits(nc):
    for fn in nc.m.functions:
        for bb in fn.blocks:
            insts = bb.instructions
            if not any(
                i.sync_info is not None and len(i.sync_info.on_wait) > 1
                for i in insts
            ):
                continue
            newlist = []
            for inst in insts:
                si = inst.sync_info
                if si is not None and len(si.on_wait) > 1:
                    waits = list(si.on_wait)
                    for w in waits[:-1]:
                        newlist.append(_fresh_nop(inst.engine, w))
                    si.on_wait = waits[-1:]
                    inst.sync_info = si
                newlist.append(inst)
            insts[:] = newlist


def _patched_drain_and_barrier(self, tick_clock, wait_clock):
    nc = self.nc
    drain_inst = nc.sync.nop(nofuse=True, hint="tail_drain_nop")
    wait_clock.add_sem_waits(
        drain_inst.ins, ScopedClock({None: tick_clock.global_clock})
    )
    nc.all_engine_barrier()
    assert self.sems is not None
    popped = nc._tile_sem_poison_stack.pop()
    assert popped is self._sem_poison
    nc.clear_and_free_semaphores(list(self.sems.allocated().values()))
    nc.all_engine_barrier()


tile.TileContext._drain_and_barrier = _patched_drain_and_barrier


# ---------------------------------------------------------------------------
def _preprocess(edge_index, N, C):
    S = N // C
    T = (S + P - 1) // P
    TP = T * P
    NP = C * TP  # total table rows (padded); PAD row index

    e = np.asarray(edge_index)
    src = np.concatenate([e[0], np.arange(N, dtype=e.dtype)]).astype(np.int64)
    dst = np.concatenate([e[1], np.arange(N, dtype=e.dtype)]).astype(np.int64)
    order = np.argsort(dst, kind="stable")
    srcs = src[order].astype(np.int64)
    dsts = dst[order]
    row_ptr = np.searchsorted(dsts, np.arange(N + 1)).astype(np.int64)
    deg = np.diff(row_ptr).astype(np.int32)

    # node n -> T1 table row (node order, per-core padded)
    t1row = (np.arange(N) // S) * TP + (np.arange(N) % S)

    perms = []
    degqs = np.ones((C, TP), np.float32)
    degns = np.ones((C, TP), np.float32)
    Ks = []
    for c in range(C):
        lo = c * S
        deg_c = deg[lo : lo + S]
        perm = np.argsort(deg_c, kind="stable")
        perms.append(perm)
        degqs[c, : S] = deg_c[perm]
        degns[c, : S] = deg_c
        Kc = []
        for t in range(T):
            seg = deg_c[perm[t * P : (t + 1) * P]]
            Kc.append(int(seg.max()) if len(seg) else 1)
        Ks.append(Kc)
    Ks = np.asarray(Ks, np.int32)
    K = Ks.max(axis=0)  # shared tile widths across cores (same program)
    SK = int(K.sum())

    # node n -> T2 table row (perm order, per-core padded)
    t2row = np.empty(N, np.int64)
    for c in range(C):
        lo = c * S
        inv = np.empty(S, np.int64)
        inv[perms[c]] = np.arange(S)
        t2row[lo : lo + S] = c * TP + inv

    slots1 = np.full((C, P, SK), NP, np.int32)
    slots2 = np.full((C, P, SK), NP, np.int32)
    for c in range(C):
        lo = c * S
        perm = perms[c]
        off = 0
        for t in range(T):
            kt = int(K[t])
            pn = perm[t * P : (t + 1) * P]
            nodes = lo + pn
            base = row_ptr[nodes]
            dg = deg[nodes]
            npn = len(pn)
            kk = np.arange(kt)
            mat = base[:, None] + kk[None, :]
            valid = kk[None, :] < dg[:, None]
            g = np.where(valid, srcs[np.minimum(mat, len(srcs) - 1)], -1)
            s1 = np.where(g >= 0, t1row[np.maximum(g, 0)], NP)
            s2 = np.where(g >= 0, t2row[np.maximum(g, 0)], NP)
            slots1[c, :npn, off : off + kt] = s1
            slots2[c, :npn, off : off + kt] = s2
            off += kt

    meta = dict(N=N, C=C, S=S, T=T, TP=TP, NP=NP,
                K=[int(k) for k in K], SK=SK)
    percore = dict(slots1=slots1, slots2=slots2, degq=degqs, degn=degns,
                   perms=perms)
    return meta, percore


def _build_program(meta, skip_gather=False, skip_coll=False, skip_phase_a=False,
                   skip_l2pe=False, skip_agg=False, skip_l1tail=False,
                   skip_l2tail=False, skip_stores=False, use_reduce=True):
    N, C, S, T, TP, NP = (meta["N"], meta["C"], meta["S"], meta["T"],
                          meta["TP"], meta["NP"])
    K, SK = meta["K"], meta["SK"]
    fp = mybir.dt.float32
    i32 = mybir.dt.int32

    ngroups = (T + GROUP - 1) // GROUP
    gmax = max(
        sum(K[g * GROUP : (g + 1) * GROUP]) for g in range(ngroups)
    )

    nc = bass.Bass("TRN2", target_bir_lowering=False, debug=False, num_devices=C)
    x_in = nc.declare_dram_parameter("x", [S, D], fp, isOutput=False)
    w1_in = nc.declare_dram_parameter("W1", [D, F1], fp, isOutput=False)
    b1_in = nc.declare_dram_parameter("b1", [1, F1], fp, isOutput=False)
    w2_in = nc.declare_dram_parameter("W2", [F1, F2], fp, isOutput=False)
    b2_in = nc.declare_dram_parameter("b2", [1, F2], fp, isOutput=False)
    degn_in = nc.declare_dram_parameter("degn", [TP], fp, isOutput=False)
    degq_in = nc.declare_dram_parameter("degq", [TP], fp, isOutput=False)
    s1_in = nc.declare_dram_parameter("slots1", [P * SK], i32, isOutput=False)
    s2_in = nc.declare_dram_parameter("slots2", [P * SK], i32, isOutput=False)
    y_out = nc.declare_dram_parameter("y", [TP, F2], fp, isOutput=True)

    q_mine = nc.dram_tensor("q_mine", [TP, F1], fp)
    u_mine = nc.dram_tensor("u_mine", [TP, F1], fp)
    T1 = nc.dram_tensor("T1", [NP + 1, F1], fp)
    T2 = nc.dram_tensor("T2", [NP + 1, F1], fp)
    groups_all = [list(range(C))]

    with tile.TileContext(nc) as tc:
        with tc.tile_pool(name="const", bufs=1) as cpool, \
             tc.tile_pool(name="xp", bufs=3) as xp, \
             tc.tile_pool(name="xtp", bufs=3) as xtp, \
             tc.tile_pool(name="ps", bufs=2, space="PSUM") as ps, \
             tc.tile_pool(name="pt", bufs=2, space="PSUM") as pt, \
             tc.tile_pool(name="pz", bufs=2, space="PSUM") as pz, \
             tc.tile_pool(name="gt", bufs=4) as gt, \
             tc.tile_pool(name="sm", bufs=4) as sm:

            ident = cpool.tile([P, P], fp)
            make_identity(nc, ident[:])
            w1s = cpool.tile([P, (D // P) * F1], fp)
            nc.sync.dma_start(
                w1s[:].rearrange("p (k f) -> p k f", f=F1),
                w1_in.ap().rearrange("(k p) f -> p k f", p=P),
            )
            # block-diagonal W2: one matmul applies W2 to GROUP tiles at once
            w2blk = cpool.tile([GROUP * F1, GROUP * F2], fp)
            nc.vector.memset(w2blk[:], 0.0)
            for i in range(GROUP):
                nc.sync.dma_start(
                    w2blk[i * F1 : (i + 1) * F1, i * F2 : (i + 1) * F2],
                    w2_in[:, :],
                )
            ones_row = cpool.tile([1, P], fp)
            nc.vector.memset(ones_row[:], 1.0)
            b1row = cpool.tile([1, F1], fp)
            nc.sync.dma_start(b1row[:], b1_in[:, :])
            b2row = cpool.tile([1, F2], fp)
            nc.sync.dma_start(b2row[:], b2_in[:, :])
            b1ps = pt.tile([P, F1], fp, space="PSUM", tag="hp")
            nc.tensor.matmul(b1ps[:], lhsT=ones_row[:], rhs=b1row[:],
                             start=True, stop=True)
            b1t = cpool.tile([P, F1], fp)
            nc.vector.tensor_copy(b1t[:], b1ps[:])
            b2ps = pz.tile([P, F2], fp, space="PSUM", tag="zp")
            nc.tensor.matmul(b2ps[:], lhsT=ones_row[:], rhs=b2row[:],
                             start=True, stop=True)
            b2t = cpool.tile([P, F2], fp)
            nc.vector.tensor_copy(b2t[:], b2ps[:])

            disn = cpool.tile([P, T], fp)
            nc.sync.dma_start(disn[:], degn_in.ap().rearrange("(t p) -> p t", p=P))
            nc.vector.reciprocal(disn[:], disn[:])
            nc.scalar.activation(disn[:], disn[:], mybir.ActivationFunctionType.Sqrt)
            disq = cpool.tile([P, T], fp)
            nc.sync.dma_start(disq[:], degq_in.ap().rearrange("(t p) -> p t", p=P))
            nc.vector.reciprocal(disq[:], disq[:])
            nc.scalar.activation(disq[:], disq[:], mybir.ActivationFunctionType.Sqrt)

            zrow = cpool.tile([1, F1], fp)
            nc.vector.memset(zrow[:], 0.0)
            nc.sync.dma_start(T1[NP : NP + 1, :], zrow[:])
            nc.sync.dma_start(T2[NP : NP + 1, :], zrow[:])

            # slot tables: one DMA each
            sl1 = cpool.tile([P, SK], i32)
            nc.sync.dma_start(sl1[:], s1_in.ap().rearrange("(p j) -> p j", j=SK))
            sl2 = cpool.tile([P, SK], i32)
            nc.sync.dma_start(sl2[:], s2_in.ap().rearrange("(p j) -> p j", j=SK))

            # staging tiles
            qstage = cpool.tile([P, T * F1], fp)
            agg1 = cpool.tile([P, T * F1], fp)
            ustage = cpool.tile([P, T * F1], fp)
            agg2 = cpool.tile([P, T * F1], fp)
            vstage = cpool.tile([P, T * F1], fp)
            zall = cpool.tile([P, T * F2], fp)
            ystage = cpool.tile([P, T * F2], fp)
            mx = cpool.tile([P, T], fp)
            nmx = cpool.tile([P, T], fp)
            se = cpool.tile([P, T], fp)
            ls = cpool.tile([P, T], fp)

            # ---------------- phase A: q = disn * (x @ W1) ----------------
            if not skip_phase_a:
                for t in range(T):
                    rows = min(P, S - t * P)
                    xt = xp.tile([P, D], fp, tag="xt")
                    if rows < P:
                        nc.vector.memset(xt[:], 0.0)
                    nc.sync.dma_start(xt[:rows, :], x_in[t * P : t * P + rows, :])
                    hp = pt.tile([P, F1], fp, space="PSUM", tag="hp")
                    for k in range(D // P):
                        tp_ = ps.tile([P, P], fp, space="PSUM", tag="tp")
                        nc.tensor.transpose(tp_[:, :], xt[:, k * P : (k + 1) * P],
                                            ident[:, :])
                        xts = xtp.tile([P, P], fp, tag="xts")
                        nc.vector.tensor_copy(xts[:, :], tp_[:, :])
                        nc.tensor.matmul(
                            hp[:, :], lhsT=xts[:, :],
                            rhs=w1s[:, k * F1 : (k + 1) * F1],
                            start=(k == 0), stop=(k == D // P - 1),
                        )
                    nc.vector.tensor_scalar(
                        qstage[:, t * F1 : (t + 1) * F1], hp[:, :],
                        disn[:, t : t + 1], None, op0=mybir.AluOpType.mult,
                    )
                if not skip_stores:
                    nc.sync.dma_start(
                        q_mine.ap().rearrange("(t p) f -> p t f", p=P),
                        qstage[:].rearrange("p (t f) -> p t f", f=F1),
                    )
            else:
                nc.vector.memset(qstage[:, 0:F1], 0.1)
                if not skip_stores:
                    nc.sync.dma_start(
                        q_mine.ap().rearrange("(t p) f -> p t f", p=P),
                        qstage[:].rearrange("p (t f) -> p t f", f=F1),
                    )

            if not skip_coll:
                nc.gpsimd.collective_compute(
                    "AllGather", mybir.AluOpType.bypass,
                    replica_groups=groups_all,
                    ins=[q_mine[:, :]], outs=[T1[0:NP, :]],
                )

            # ---------------- gather + aggregate ----------------
            def agg_phase_accum(table, sl, aggt):
                off = 0
                for t in range(T):
                    kt = K[t]
                    for k in range(kt):
                        nc.gpsimd.indirect_dma_start(
                            out=aggt[:, t * F1 : (t + 1) * F1],
                            out_offset=None,
                            in_=table[:, :],
                            in_offset=bass.IndirectOffsetOnAxis(
                                ap=sl[:, off + k : off + k + 1], axis=0
                            ),
                            compute_op=(mybir.AluOpType.bypass if k == 0
                                        else mybir.AluOpType.add),
                        )
                    off += kt

            def agg_phase_reduce(table, sl, aggt):
                off = 0
                for g in range(ngroups):
                    ts = range(g * GROUP, min((g + 1) * GROUP, T))
                    kg = sum(K[t] for t in ts)
                    gtile = gt.tile([P, gmax * F1], fp, tag="gt")
                    if not skip_gather:
                        for j in range(kg):
                            nc.gpsimd.indirect_dma_start(
                                out=gtile[:, j * F1 : (j + 1) * F1],
                                out_offset=None,
                                in_=table[:, :],
                                in_offset=bass.IndirectOffsetOnAxis(
                                    ap=sl[:, off + j : off + j + 1], axis=0
                                ),
                            )
                    else:
                        nc.vector.memset(gtile[:, 0 : kg * F1], 0.5)
                    ct = 0
                    for t in ts:
                        kt = K[t]
                        nc.vector.tensor_reduce(
                            out=aggt[:, t * F1 : (t + 1) * F1, None],
                            in_=gtile[:, ct * F1 : (ct + kt) * F1].rearrange(
                                "p (k f) -> p f k", f=F1
                            ),
                            op=mybir.AluOpType.add, axis=mybir.AxisListType.X,
                        )
                        ct += kt
                    off += kg

            agg_phase = agg_phase_reduce if use_reduce else agg_phase_accum
            if skip_agg or skip_gather:
                nc.vector.memset(agg1[:], 0.5)
            else:
                agg_phase(T1, sl1, agg1)

            # ---------------- L1 tail: u = relu(disq*agg + b1)*disq -------
            if skip_l1tail:
                nc.vector.memset(ustage[:], 0.25)
            for t in ([] if skip_l1tail else range(T)):
                tmp = sm.tile([P, F1], fp, tag="tmp1")
                nc.vector.scalar_tensor_tensor(
                    out=tmp[:], in0=agg1[:, t * F1 : (t + 1) * F1],
                    scalar=disq[:, t : t + 1], in1=b1t[:],
                    op0=mybir.AluOpType.mult, op1=mybir.AluOpType.add,
                )
                nc.vector.tensor_scalar(
                    ustage[:, t * F1 : (t + 1) * F1], tmp[:], 0.0,
                    disq[:, t : t + 1],
                    op0=mybir.AluOpType.max, op1=mybir.AluOpType.mult,
                )
            if not skip_stores:
                nc.sync.dma_start(
                    u_mine.ap().rearrange("(t p) f -> p t f", p=P),
                    ustage[:].rearrange("p (t f) -> p t f", f=F1),
                )

            if not skip_coll:
                nc.gpsimd.collective_compute(
                    "AllGather", mybir.AluOpType.bypass,
                    replica_groups=groups_all,
                    ins=[u_mine[:, :]], outs=[T2[0:NP, :]],
                )

            if skip_agg or skip_gather:
                nc.vector.memset(agg2[:], 0.5)
            else:
                agg_phase(T2, sl2, agg2)

            # ---------------- L2 tail ------------------------------------
            if skip_l2tail:
                nc.vector.memset(vstage[:], 0.25)
            for t in ([] if skip_l2tail else range(T)):
                nc.vector.tensor_scalar(
                    vstage[:, t * F1 : (t + 1) * F1],
                    agg2[:, t * F1 : (t + 1) * F1],
                    disq[:, t : t + 1], None, op0=mybir.AluOpType.mult,
                )
            if not skip_l2pe:
                for g in range(ngroups):
                    ts = list(range(g * GROUP, min((g + 1) * GROUP, T)))
                    nt = len(ts)
                    vtp = ps.tile([P, P], fp, space="PSUM", tag="tp")
                    nc.tensor.transpose(
                        vtp[: nt * F1, :],
                        vstage[:, ts[0] * F1 : (ts[0] + nt) * F1], ident[:, :],
                    )
                    vts = xtp.tile([P, P], fp, tag="xts")
                    nc.vector.tensor_copy(vts[: nt * F1, :], vtp[: nt * F1, :])
                    zps = pz.tile([P, GROUP * F2], fp, space="PSUM", tag="zp")
                    nc.tensor.matmul(
                        zps[:, : nt * F2],
                        lhsT=vts[: nt * F1, :],
                        rhs=w2blk[: nt * F1, : nt * F2],
                        start=True, stop=True,
                    )
                    nc.vector.tensor_copy(
                        zall[:, ts[0] * F2 : (ts[0] + nt) * F2],
                        zps[:, : nt * F2],
                    )
            else:
                nc.vector.memset(zall[:], 0.5)

            for t in ([] if skip_l2tail else range(T)):
                nc.vector.tensor_add(
                    zall[:, t * F2 : (t + 1) * F2],
                    zall[:, t * F2 : (t + 1) * F2], b2t[:],
                )
            if not skip_l2tail:
                nc.vector.tensor_reduce(
                    out=mx[:, :, None],
                    in_=zall[:].rearrange("p (t f) -> p t f", f=F2),
                    op=mybir.AluOpType.max, axis=mybir.AxisListType.X,
                )
                nc.vector.tensor_scalar_mul(nmx[:], mx[:], -1.0)
            for t in ([] if skip_l2tail else range(T)):
                nc.scalar.activation(
                    ystage[:, t * F2 : (t + 1) * F2],
                    zall[:, t * F2 : (t + 1) * F2],
                    mybir.ActivationFunctionType.Exp,
                    bias=nmx[:, t : t + 1], accum_out=se[:, t : t + 1],
                )
            if not skip_l2tail:
                nc.scalar.activation(ls[:], se[:], mybir.ActivationFunctionType.Ln)
            if skip_l2tail:
                nc.vector.memset(ystage[:], 0.25)
            for t in ([] if skip_l2tail else range(T)):
                nc.vector.tensor_scalar(
                    ystage[:, t * F2 : (t + 1) * F2],
                    zall[:, t * F2 : (t + 1) * F2],
                    mx[:, t : t + 1], ls[:, t : t + 1],
                    op0=mybir.AluOpType.subtract, op1=mybir.AluOpType.subtract,
                )
            nc.sync.dma_start(
                y_out.ap().rearrange("(t p) f -> p t f", p=P),
                ystage[:].rearrange("p (t f) -> p t f", f=F2),
            )

    _split_multi_waits(nc)
    return nc


# ---------------------------------------------------------------------------
class _Runner:
    def __init__(self, nc, n_cores):
        import jax
        from jax.sharding import Mesh, PartitionSpec
        from jax.experimental.shard_map import shard_map
        from concourse.bass2jax import (
            _bass_exec_p, partition_id_tensor, install_neuronx_cc_hook,
        )

        install_neuronx_cc_hook()
        self.jax = jax
        self.n_cores = n_cores
        in_names, out_names, out_avals = [], [], []
        partition_name = (
            nc.partition_id_tensor.name if nc.partition_id_tensor else None
        )
        for alloc in nc.m.functions[0].allocations:
            if not isinstance(alloc, mybir.MemoryLocationSet):
                continue
            name = alloc.memorylocations[0].name
            if alloc.kind == "ExternalInput":
                if name != partition_name:
                    in_names.append(name)
            elif alloc.kind == "ExternalOutput":
                out_names.append(name)
                out_avals.append(
                    jax.core.ShapedArray(
                        tuple(alloc.tensor_shape), mybir.dt.np(alloc.dtype)
                    )
                )
        self.in_names, self.out_names, self.out_avals = in_names, out_names, out_avals
        n_params, n_outs = len(in_names), len(out_avals)
        all_in = in_names + out_names
        if partition_name is not None:
            all_in.append(partition_name)

        def _body(*args):
            operands = list(args)
            if partition_name is not None:
                operands.append(partition_id_tensor())
            return tuple(
                _bass_exec_p.bind(
                    *operands, out_avals=tuple(out_avals), in_names=tuple(all_in),
                    out_names=tuple(out_names), lowering_input_output_aliases=(),
                    sim_require_finite=True, sim_require_nnan=True, nc=nc,
                )
            )

        devices = jax.devices()[:n_cores]
        mesh = Mesh(np.asarray(devices), ("core",))
        self.fn = jax.jit(
            shard_map(
                _body, mesh=mesh,
                in_specs=(PartitionSpec("core"),) * (n_params + n_outs),
                out_specs=(PartitionSpec("core"),) * n_outs,
                check_rep=False,
            ),
            keep_unused=True,
        )

    def run(self, in_maps):
        concat = [
            np.concatenate([np.asarray(m[name]) for m in in_maps], axis=0)
            for name in self.in_names
        ]
        zeros = [
            np.zeros((self.n_cores * a.shape[0], *a.shape[1:]), a.dtype)
            for a in self.out_avals
        ]
        out = self.fn(*concat, *zeros)
        self.jax.block_until_ready(out)
        res = []
        for c in range(self.n_cores):
            res.append({
                name: np.asarray(out[i]).reshape(
                    self.n_cores, *self.out_avals[i].shape
                )[c]
                for i, name in enumerate(self.out_names)
            })
        return res


_CACHE = {}


def _make_in_maps(inputs, meta, percore):
    C, S = meta["C"], meta["S"]
    x = np.asarray(inputs["x"], np.float32)
    in_maps = []
    for c in range(C):
        in_maps.append({
            "x": x[c * S : (c + 1) * S],
            "W1": np.asarray(inputs["W1"], np.float32),
            "b1": np.asarray(inputs["b1"], np.float32)[None],
            "W2": np.asarray(inputs["W2"], np.float32),
            "b2": np.asarray(inputs["b2"], np.float32)[None],
            "degn": percore["degn"][c], "degq": percore["degq"][c],
            "slots1": percore["slots1"][c].reshape(-1),
            "slots2": percore["slots2"][c].reshape(-1),
        })
    return in_maps


def kernel(x, edge_index, W1, b1, W2, b2, _build_kw=None):
    inputs = dict(x=x, W1=W1, b1=b1, W2=W2, b2=b2)
    N, C = np.asarray(x).shape[0], N_CORES
    S = N // C

    meta, percore = _preprocess(edge_index, N, C)
    key = ("gcn2", tuple(meta["K"]), tuple(sorted((_build_kw or {}).items())))
    if key not in _CACHE:
        nc = _build_program(meta, **(_build_kw or {}))
        _CACHE[key] = _Runner(nc, C)
    runner = _CACHE[key]

    in_maps = _make_in_maps(inputs, meta, percore)
    res = runner.run(in_maps)

    y = np.empty((N, F2), np.float32)
    for c in range(C):
        y[c * S + percore["perms"][c]] = res[c]["y"][:S]
    return y
